# revision 19
# baseline (speedup 1.0000x reference)
"""Trainium2 Bass kernel for CompositionalResidualMLP (MoE routing, 2-node module network).

Strategy: data-parallel over batch across 8 NeuronCores. On the host, samples are
dealt round-robin (per routing pair) to cores and sorted into 64 (a0,a1) blocks
per core with per-a1 block capacities, so every layer is a dense per-module
matmul: node-0 layers see contiguous column ranges per a0 module, node-1 layers
see regular-strided column blocks per a1 module (3-level access patterns).
Activations live feature-major in SBUF ([features, samples]), so no transposes
are needed anywhere. Matmuls run as float32r (full-rate fp32 at N>=256) with
fused bias+ReLU evictions alternating between the Scalar and Vector engines.
Input DMAs are chunked and ordered so the first matmuls start as soon as the
first module block lands; a short bf16 matmul burst warms the PE clock gate
during the DMA lead-in.
"""

import numpy as np
from contextlib import ExitStack

# Problem constants (hardcoded per contract)
B_TOT = 32768
D0 = 64
D1 = 64
M = 8          # modules per node
H = 256        # hidden width
O0 = 128
O1 = 32
NCORES = 8

USE_BF16 = True    # matmul/act dtype: False -> fp32 data with float32r matmuls
WARMUP_MMS = 10    # bf16 warm-up matmuls at kernel start (HAM clock-gate warm)


def _build_bass(S, Cj, off, ncol):
    """Build the per-core Bass program. All cores run the identical program on
    different data (pure SPMD, no collectives)."""
    import concourse.bass as bass
    import concourse.tile as tile
    from concourse import bacc, mybir

    F32 = mybir.dt.float32
    F32R = mybir.dt.float32r
    BF16 = mybir.dt.bfloat16
    Relu = mybir.ActivationFunctionType.Relu
    Identity = mybir.ActivationFunctionType.Identity
    Add = mybir.AluOpType.add
    Max = mybir.AluOpType.max

    DT = BF16 if USE_BF16 else F32          # storage dtype of matmul operands
    cast = (lambda ap: ap) if USE_BF16 else (lambda ap: ap.bitcast(F32R))

    nc = bacc.Bacc("TRN2", target_bir_lowering=False, debug=False,
                   enable_asserts=False, num_devices=NCORES)

    # DRAM I/O
    x0t = nc.dram_tensor("x0t", [128, ncol // 2], DT, kind="ExternalInput").ap()
    x1t = nc.dram_tensor("x1t", [128, ncol // 2], DT, kind="ExternalInput").ap()
    w00 = nc.dram_tensor("w00", [128, M * H // 2], DT, kind="ExternalInput").ap()
    w01 = nc.dram_tensor("w01", [128, 2 * M * O0], DT, kind="ExternalInput").ap()
    w1p = nc.dram_tensor("w1p", [128, M * H], DT, kind="ExternalInput").ap()
    w1a = nc.dram_tensor("w1a", [128, 3 * M * O0], DT, kind="ExternalInput").ap()
    w1o = nc.dram_tensor("w1o", [128, M * O1], DT, kind="ExternalInput").ap()
    b00 = nc.dram_tensor("b00", [128, 2 * M], F32, kind="ExternalInput").ap()
    b01 = nc.dram_tensor("b01", [128, M], F32, kind="ExternalInput").ap()
    b1p = nc.dram_tensor("b1p", [128, 2 * M], F32, kind="ExternalInput").ap()
    b1a = nc.dram_tensor("b1a", [128, M], F32, kind="ExternalInput").ap()
    b1o = nc.dram_tensor("b1o", [O1, M], F32, kind="ExternalInput").ap()
    outT = nc.dram_tensor("outT", [O1, ncol], F32, kind="ExternalOutput").ap()

    def n_splits(total, cap=512):
        ns = -(-total // cap)
        if total % 2 == 0:
            # even chunk sizes (fp32r ifmap needs even innermost counts)
            half = total // 2
            b2 = half // ns
            r2 = half - b2 * ns
            sizes = [2 * (b2 + (1 if k < r2 else 0)) for k in range(ns)]
        else:
            base = total // ns
            rem = total - base * ns
            sizes = [base + (1 if k < rem else 0) for k in range(ns)]
        outs = []
        pos = 0
        for sz in sizes:
            outs.append((pos, sz))
            pos += sz
        return outs

    a0_splits = n_splits(S)              # contiguous column splits within a module's S columns
    assert len(a0_splits) == 2 and a0_splits[0][1] == a0_splits[1][1], \
        "merged 2-bank evictions assume two equal a0 splits"
    blk_splits = n_splits(M, 4)          # i-block splits for a1-routed layers (4 blocks each)

    with tile.TileContext(nc) as tc:
        with ExitStack() as ctx:
            acts = ctx.enter_context(tc.tile_pool(name="acts", bufs=1))
            wpool = ctx.enter_context(tc.tile_pool(name="w", bufs=1))
            ps128 = ctx.enter_context(tc.tile_pool(name="ps128", bufs=4, space="PSUM"))

            # ---- HAM warm-up: short bf16 matmul burst on scratch data keeps the
            # PE clock gate at 8/8 while input DMAs stream in. fp32r matmuls do
            # not register as PE activity for the clock gate, so we also emit a
            # tiny bf16 matmul every few real matmuls to hold it at 8/8.
            wu = None
            if WARMUP_MMS:
                wu = wpool.tile([128, 512], BF16, tag="wu")
                nc.vector.memset(wu[:], 0.0)
                for _ in range(WARMUP_MMS):
                    pw = ps128.tile([128, 512], F32, tag="ps")
                    nc.tensor.matmul(pw[:], wu[:, 0:128], wu[:], start=True, stop=True)

            # weights + biases to SBUF
            w00s = wpool.tile([128, M * H // 2], DT, tag="w00")
            w01s = wpool.tile([128, 2 * M * O0], DT, tag="w01")
            w1ps = wpool.tile([128, M * H], DT, tag="w1p")
            w1as = wpool.tile([128, 3 * M * O0], DT, tag="w1a")
            w1os = wpool.tile([128, M * O1], DT, tag="w1o")
            b00s = wpool.tile([128, 2 * M], F32, tag="b00")
            b01s = wpool.tile([128, M], F32, tag="b01")
            b1ps = wpool.tile([128, 2 * M], F32, tag="b1p")
            b1as = wpool.tile([128, M], F32, tag="b1a")
            b1os = wpool.tile([O1, M], F32, tag="b1o")

            x0s = acts.tile([128, ncol // 2], DT, tag="x0")
            x1s = acts.tile([128, ncol // 2], DT, tag="x1")
            h1a = acts.tile([128, ncol], DT, tag="h1a")
            h1b = acts.tile([128, ncol], DT, tag="h1b")
            hs = acts.tile([128, ncol], DT, tag="h")
            g1a = acts.tile([128, ncol], DT, tag="g1a")
            g1b = acts.tile([128, ncol], DT, tag="g1b")

            # DMA order = consumption order; x0 split per module so L1 i=0 can
            # start after ~0.7 MB instead of after the whole input set.
            nc.sync.dma_start(cast(w00s[:]), cast(w00))
            nc.sync.dma_start(b00s[:], b00)
            for i in range(M // 2):
                nc.sync.dma_start(cast(x0s[:, i * S:(i + 1) * S]),
                                  cast(x0t[:, i * S:(i + 1) * S]))
            # enqueue the node-1 inputs from the (idle) GpSimd queue in
            # parallel with Sync's x0 chunks, so x1/w1p transfers start early
            nc.gpsimd.dma_start(cast(w1ps[:]), cast(w1p))
            nc.gpsimd.dma_start(b1ps[:], b1p)
            nc.gpsimd.dma_start(cast(x1s[:]), cast(x1t))
            nc.gpsimd.dma_start(cast(w01s[:]), cast(w01))
            nc.gpsimd.dma_start(b01s[:], b01)
            nc.gpsimd.dma_start(cast(w1as[:]), cast(w1a))
            nc.gpsimd.dma_start(b1as[:], b1a)
            nc.gpsimd.dma_start(cast(w1os[:]), cast(w1o))
            nc.gpsimd.dma_start(b1os[:], b1o)

            parity = 0

            def evict(dst_ap, psum_ap, bias_ap, relu):
                nonlocal parity
                if relu and not USE_BF16:
                    # these tensors feed later fp32r matmuls -> write as fp32r
                    dst_ap = dst_ap.bitcast(F32R)
                if parity == 0:
                    if relu:
                        nc.scalar.activation(dst_ap, psum_ap, Relu, bias=bias_ap)
                    else:
                        nc.scalar.activation(dst_ap, psum_ap, Identity, bias=bias_ap)
                else:
                    if relu:
                        nc.vector.tensor_scalar(dst_ap, psum_ap, bias_ap, 0.0, Add, Max)
                    else:
                        nc.vector.tensor_scalar_add(dst_ap, psum_ap, bias_ap)
                parity ^= 1

            def strided(tensor_tile, j, b0, nb, cj):
                # columns off[j] + i*S + [0, cj) for i in [b0, b0+nb)
                v = tensor_tile[:].rearrange("p (i c) -> p i c", i=M)
                return v[:, b0:b0 + nb, off[j]:off[j] + cj]

            # ---- L1: h1 = relu(W00[a0].T @ x0 + b00[a0])  [H=256 -> 2 chunks]
            # modules ip and ip+4 run concurrently on PE row-halves (K=64 each)
            # two N-splits land in the two banks of one PSUM tile; one evict covers both
            def bank2(pt, sizes):
                v = pt[:].rearrange("p (b c) -> p b c", b=2)
                return [v[:, s, 0:sz] for s, (pos, sz) in enumerate(sizes)]

            for ip in range(M // 2):
                for mo in range(2):
                    dst_tile = h1a if mo == 0 else h1b
                    wcols = slice(ip * H + mo * 128, ip * H + (mo + 1) * 128)
                    for half, base, bia in ((slice(0, 64), ip, mo * M + ip),
                                            (slice(64, 128), ip + 4, mo * M + ip + 4)):
                        pt = ps128.tile([128, 1024], F32, tag="ps")
                        outs2 = bank2(pt, a0_splits)
                        for s, (pos, sz) in enumerate(a0_splits):
                            # both partition-halves read columns ip*S..: module
                            # ip+4's samples sit on partitions 64-127 at the
                            # same column offsets as module ip
                            nc.tensor.matmul(outs2[s], cast(w00s[half, wcols]),
                                             cast(x0s[half, ip * S + pos: ip * S + pos + sz]),
                                             start=True, stop=True)
                        sz0 = a0_splits[0][1]
                        src_ap = pt[:].rearrange("p (b c) -> p b c", b=2)[:, :, 0:sz0]
                        dst_ap = dst_tile[:, base * S: base * S + S].rearrange(
                            "p (b c) -> p b c", b=2)
                        evict(dst_ap, src_ap, b00s[:, bia: bia + 1], True)

            # ---- L3: g1 = relu(W1p[a1].T @ x1 + b1p[a1])  (emitted early: only needs x1)
            # block-halves i=0..3 / i=4..7 run concurrently on PE row-halves
            def strided_half(tile_t, part0, j, cj):
                v = tile_t[:].rearrange("p (i c) -> p i c", i=M // 2)
                return v[part0:part0 + 64, :, off[j]:off[j] + cj]

            for j in range(M):
                cj = Cj[j]
                for mo in range(2):
                    dst_tile = g1a if mo == 0 else g1b
                    wcols = slice(j * H + mo * 128, j * H + (mo + 1) * 128)
                    pt = ps128.tile([128, 1024], F32, tag="ps")
                    ptb = pt[:].rearrange("p (b c) -> p b c", b=2)
                    nc.tensor.matmul(ptb[:, 0, 0:4 * cj].rearrange("p (i c) -> p i c", c=cj),
                                     cast(w1ps[0:64, wcols]),
                                     cast(strided_half(x1s, 0, j, cj)),
                                     start=True, stop=True)
                    nc.tensor.matmul(ptb[:, 1, 0:4 * cj].rearrange("p (i c) -> p i c", c=cj),
                                     cast(w1ps[64:128, wcols]),
                                     cast(strided_half(x1s, 64, j, cj)),
                                     start=True, stop=True)
                    src_ap = ptb[:, :, 0:4 * cj].rearrange("p b (i c) -> p b i c", c=cj)
                    evict(strided(dst_tile, j, 0, 8, cj), src_ap,
                          b1ps[:, mo * M + j: mo * M + j + 1], True)

            # ---- L2: h = relu(W01[a0].T @ h1 + b01[a0])  [K=256 -> 2 accum chunks]
            for i in range(M):
                pt = ps128.tile([128, 1024], F32, tag="ps")
                outs2 = bank2(pt, a0_splits)
                for s, (pos, sz) in enumerate(a0_splits):
                    nc.tensor.matmul(
                        outs2[s],
                        cast(w01s[:, (0 * M + i) * O0: (0 * M + i + 1) * O0]),
                        cast(h1a[:, i * S + pos: i * S + pos + sz]),
                        start=True, stop=False)
                    nc.tensor.matmul(
                        outs2[s],
                        cast(w01s[:, (1 * M + i) * O0: (1 * M + i + 1) * O0]),
                        cast(h1b[:, i * S + pos: i * S + pos + sz]),
                        start=False, stop=True)
                sz0 = a0_splits[0][1]
                src_ap = pt[:].rearrange("p (b c) -> p b c", b=2)[:, :, 0:sz0]
                dst_ap = hs[:, i * S: i * S + S].rearrange("p (b c) -> p b c", b=2)
                evict(dst_ap, src_ap, b01s[:, i: i + 1], True)

            # ---- L4: g = relu(W1a[a1].T @ concat(h, g1) + b1a[a1]) [K=384 -> 3 accum chunks]
            # g reuses h1a's slot (h1 is dead after L2)
            gs = acts.tile([128, ncol], DT, tag="h1a")
            for j in range(M):
                cj = Cj[j]
                pt = ps128.tile([128, 1024], F32, tag="ps")
                ptb = pt[:].rearrange("p (b c) -> p b c", b=2)
                for s, (b0, nb) in enumerate(blk_splits):
                    ptv = ptb[:, s, 0:nb * cj].rearrange("p (i c) -> p i c", c=cj)
                    for kc, src in enumerate((hs, g1a, g1b)):
                        nc.tensor.matmul(
                            ptv,
                            cast(w1as[:, (kc * M + j) * O0: (kc * M + j + 1) * O0]),
                            cast(strided(src, j, b0, nb, cj)),
                            start=(kc == 0), stop=(kc == 2))
                src_ap = ptb[:, :, 0:4 * cj].rearrange("p b (i c) -> p b i c", c=cj)
                evict(strided(gs, j, 0, 8, cj), src_ap,
                      b1as[:, j: j + 1], True)

            # ---- L5: out = W1o[a1].T @ g + b1o[a1]  (identity)
            # outT reuses h1b's slot; written + DMA'd out per module j
            outs = acts.tile([O1, ncol], F32, tag="h1b")
            for j in range(M):
                cj = Cj[j]
                pt = ps128.tile([O1, 1024], F32, tag="ps")
                ptb = pt[:].rearrange("p (b c) -> p b c", b=2)
                for s, (b0, nb) in enumerate(blk_splits):
                    ptv = ptb[:, s, 0:nb * cj].rearrange("p (i c) -> p i c", c=cj)
                    nc.tensor.matmul(
                        ptv,
                        cast(w1os[:, j * O1: (j + 1) * O1]),
                        cast(strided(gs, j, b0, nb, cj)),
                        start=True, stop=True)
                src_ap = ptb[:, :, 0:4 * cj].rearrange("p b (i c) -> p b i c", c=cj)
                evict(strided(outs, j, 0, 8, cj), src_ap,
                      b1os[:, j: j + 1], False)
                # stream this module's output columns back while later modules compute
                vout = outs[:].rearrange("p (i c) -> p i c", i=M)[:, :, off[j]:off[j] + cj]
                vdst = outT.rearrange("p (i c) -> p i c", i=M)[:, :, off[j]:off[j] + cj]
                nc.sync.dma_start(vdst, vout)

    nc.compile()
    return nc


def _pack_weights(inputs):
    """Pack per-module weight stacks into SBUF-image layouts (contraction dim on
    partitions, [K<=128, chunks*modules*out] on the free axis)."""
    import ml_dtypes
    wdt = ml_dtypes.bfloat16 if USE_BF16 else np.float32
    f = lambda a: np.ascontiguousarray(a.astype(wdt))
    g = lambda a: np.ascontiguousarray(a.astype(np.float32))
    W00 = inputs["W00"]; W01 = inputs["W01"]; W1p = inputs["W1p"]
    W1a = inputs["W1a"]; W1o = inputs["W1o"]
    w00lo = W00[:M // 2].transpose(1, 0, 2).reshape(D0, M * H // 2)
    w00hi = W00[M // 2:].transpose(1, 0, 2).reshape(D0, M * H // 2)
    w1pp = W1p.transpose(1, 0, 2).reshape(D1, M * H)
    return {
        "w00": f(np.concatenate([w00lo, w00hi], axis=0)),
        "w01": f(W01.reshape(M, 2, 128, O0).transpose(2, 1, 0, 3).reshape(128, 2 * M * O0)),
        "w1p": f(np.concatenate([w1pp, w1pp], axis=0)),
        "w1a": f(W1a.reshape(M, 3, 128, O0).transpose(2, 1, 0, 3).reshape(128, 3 * M * O0)),
        "w1o": f(W1o.transpose(1, 0, 2).reshape(128, M * O1)),
        "b00": g(inputs["b00"].reshape(M, 2, 128).transpose(2, 1, 0).reshape(128, 2 * M)),
        "b01": g(inputs["b01"].T),
        "b1p": g(inputs["b1p"].reshape(M, 2, 128).transpose(2, 1, 0).reshape(128, 2 * M)),
        "b1a": g(inputs["b1a"].T),
        "b1o": g(inputs["b1o"].T),
    }


def _route(input_val):
    """Assign each sample to a (core, column) in the blocked layout."""
    a0 = np.argmax(input_val[:, D0 + D1: D0 + D1 + M], axis=1)
    a1 = np.argmax(input_val[:, D0 + D1 + M: D0 + D1 + 2 * M], axis=1)
    B = input_val.shape[0]
    nij = np.zeros((M, M), dtype=np.int64)
    np.add.at(nij, (a0, a1), 1)
    # capacities rounded up to even: fp32r matmul ifmap needs even innermost counts
    Cj = np.maximum((-(-nij.max(axis=0) // NCORES) + 1) // 2 * 2, 64)
    off = np.concatenate([[0], np.cumsum(Cj)[:-1]]).astype(np.int64)
    S = int(Cj.sum())
    ncol = M * S

    pairkey = a0 * M + a1
    order = np.argsort(pairkey, kind="stable")
    counts = np.bincount(pairkey, minlength=M * M)
    group_start = np.concatenate([[0], np.cumsum(counts)[:-1]])
    rank_sorted = np.arange(B) - np.repeat(group_start, counts)
    rank = np.empty(B, dtype=np.int64)
    rank[order] = rank_sorted
    core = rank % NCORES
    slot = rank // NCORES
    assert np.all(slot < Cj[a1]), "capacity overflow"
    col = a0 * S + off[a1] + slot
    return core, col, S, [int(c) for c in Cj], [int(o) for o in off], ncol


def kernel(**inputs):
    import os
    import ml_dtypes
    from concourse.bass_utils import run_bass_kernel_spmd

    input_val = np.asarray(inputs["input_val"], dtype=np.float32)
    B = input_val.shape[0]

    core, col, S, Cj, off, ncol = _route(input_val)

    xdt = ml_dtypes.bfloat16 if USE_BF16 else np.float32
    feat0 = input_val[:, :D0]
    feat1 = input_val[:, D0:D0 + D1]
    # split layout: module blocks i<4 (by a0) on partitions 0-63, i>=4 on 64-127
    half = ncol // 2
    X0T = np.zeros((NCORES, 128, half), dtype=xdt)
    X1T = np.zeros((NCORES, 128, half), dtype=xdt)
    hi = col >= half
    prow = np.where(hi, 64, 0)
    pcol = np.where(hi, col - half, col)
    for r in (0, 64):
        m = prow == r
        X0T[core[m], r:r + 64, pcol[m]] = feat0[m].astype(xdt)
        X1T[core[m], r:r + 64, pcol[m]] = feat1[m].astype(xdt)

    wmap = _pack_weights({k: np.asarray(v, dtype=np.float32) for k, v in inputs.items()
                          if k != "input_val"})

    nc = _build_bass(S, Cj, off, ncol)

    in_maps = [dict(wmap, x0t=np.ascontiguousarray(X0T[c]),
                    x1t=np.ascontiguousarray(X1T[c])) for c in range(NCORES)]
    res = run_bass_kernel_spmd(nc, in_maps, core_ids=list(range(NCORES)),
                               tmpdir=os.environ.get("BASS_TMPDIR"))
    global _LAST_RESULTS
    _LAST_RESULTS = res

    OUT = np.stack([r["outT"] for r in res.results])  # [NCORES, O1, ncol]
    return np.ascontiguousarray(OUT[core, :, col]).astype(np.float32)


# revision 20
# speedup vs baseline: 1.0641x; 1.0641x over previous
"""Trainium2 Bass kernel for CompositionalResidualMLP (MoE routing, 2-node module network).

Strategy: data-parallel over batch across 8 NeuronCores. On the host, samples are
dealt round-robin (per routing pair) to cores and sorted into 64 (a0,a1) blocks
per core with per-a1 block capacities, so every layer is a dense per-module
matmul: node-0 layers see contiguous column ranges per a0 module, node-1 layers
see regular-strided column blocks per a1 module (3-level access patterns).
Activations live feature-major in SBUF ([features, samples]), so no transposes
are needed anywhere. Matmuls run as float32r (full-rate fp32 at N>=256) with
fused bias+ReLU evictions alternating between the Scalar and Vector engines.
Input DMAs are chunked and ordered so the first matmuls start as soon as the
first module block lands; a short bf16 matmul burst warms the PE clock gate
during the DMA lead-in.
"""

import numpy as np
from contextlib import ExitStack

# Problem constants (hardcoded per contract)
B_TOT = 32768
D0 = 64
D1 = 64
M = 8          # modules per node
H = 256        # hidden width
O0 = 128
O1 = 32
NCORES = 8

USE_BF16 = True    # matmul/act dtype: False -> fp32 data with float32r matmuls
WARMUP_MMS = 10    # bf16 warm-up matmuls at kernel start (HAM clock-gate warm)


def _build_bass(S, Cj, off, ncol):
    """Build the per-core Bass program. All cores run the identical program on
    different data (pure SPMD, no collectives)."""
    import concourse.bass as bass
    import concourse.tile as tile
    from concourse import bacc, mybir

    F32 = mybir.dt.float32
    F32R = mybir.dt.float32r
    BF16 = mybir.dt.bfloat16
    Relu = mybir.ActivationFunctionType.Relu
    Identity = mybir.ActivationFunctionType.Identity
    Add = mybir.AluOpType.add
    Max = mybir.AluOpType.max

    DT = BF16 if USE_BF16 else F32          # storage dtype of matmul operands
    cast = (lambda ap: ap) if USE_BF16 else (lambda ap: ap.bitcast(F32R))

    nc = bacc.Bacc("TRN2", target_bir_lowering=False, debug=False,
                   enable_asserts=False, num_devices=NCORES)

    # DRAM I/O
    x0t = nc.dram_tensor("x0t", [128, ncol // 2], DT, kind="ExternalInput").ap()
    x1t = nc.dram_tensor("x1t", [128, ncol // 2], DT, kind="ExternalInput").ap()
    w00 = nc.dram_tensor("w00", [128, M * H // 2], DT, kind="ExternalInput").ap()
    w01 = nc.dram_tensor("w01", [128, 2 * M * O0], DT, kind="ExternalInput").ap()
    w1p = nc.dram_tensor("w1p", [128, M * H], DT, kind="ExternalInput").ap()
    w1a = nc.dram_tensor("w1a", [128, 3 * M * O0], DT, kind="ExternalInput").ap()
    w1o = nc.dram_tensor("w1o", [128, M * O1], DT, kind="ExternalInput").ap()
    b00 = nc.dram_tensor("b00", [128, 2 * M], F32, kind="ExternalInput").ap()
    b01 = nc.dram_tensor("b01", [128, M], F32, kind="ExternalInput").ap()
    b1p = nc.dram_tensor("b1p", [128, 2 * M], F32, kind="ExternalInput").ap()
    b1a = nc.dram_tensor("b1a", [128, M], F32, kind="ExternalInput").ap()
    b1o = nc.dram_tensor("b1o", [O1, M], F32, kind="ExternalInput").ap()
    outT = nc.dram_tensor("outT", [O1, ncol], F32, kind="ExternalOutput").ap()

    def n_splits(total, cap=512):
        ns = -(-total // cap)
        if total % 2 == 0:
            # even chunk sizes (fp32r ifmap needs even innermost counts)
            half = total // 2
            b2 = half // ns
            r2 = half - b2 * ns
            sizes = [2 * (b2 + (1 if k < r2 else 0)) for k in range(ns)]
        else:
            base = total // ns
            rem = total - base * ns
            sizes = [base + (1 if k < rem else 0) for k in range(ns)]
        outs = []
        pos = 0
        for sz in sizes:
            outs.append((pos, sz))
            pos += sz
        return outs

    a0_splits = n_splits(S)              # contiguous column splits within a module's S columns
    assert len(a0_splits) == 2 and a0_splits[0][1] == a0_splits[1][1], \
        "merged 2-bank evictions assume two equal a0 splits"
    blk_splits = n_splits(M, 4)          # i-block splits for a1-routed layers (4 blocks each)

    with tile.TileContext(nc) as tc:
        with ExitStack() as ctx:
            acts = ctx.enter_context(tc.tile_pool(name="acts", bufs=1))
            wpool = ctx.enter_context(tc.tile_pool(name="w", bufs=1))
            ps128 = ctx.enter_context(tc.tile_pool(name="ps128", bufs=4, space="PSUM"))

            # ---- HAM warm-up: short bf16 matmul burst on scratch data keeps the
            # PE clock gate at 8/8 while input DMAs stream in. fp32r matmuls do
            # not register as PE activity for the clock gate, so we also emit a
            # tiny bf16 matmul every few real matmuls to hold it at 8/8.
            wu = None
            if WARMUP_MMS:
                wu = wpool.tile([128, 512], BF16, tag="wu")
                nc.vector.memset(wu[:], 0.0)
                for _ in range(WARMUP_MMS):
                    pw = ps128.tile([128, 512], F32, tag="ps")
                    nc.tensor.matmul(pw[:], wu[:, 0:128], wu[:], start=True, stop=True)

            # weights + biases to SBUF
            w00s = wpool.tile([128, M * H // 2], DT, tag="w00")
            w01s = wpool.tile([128, 2 * M * O0], DT, tag="w01")
            w1ps = wpool.tile([128, M * H], DT, tag="w1p")
            w1as = wpool.tile([128, 3 * M * O0], DT, tag="w1a")
            w1os = wpool.tile([128, M * O1], DT, tag="w1o")
            b00s = wpool.tile([128, 2 * M], F32, tag="b00")
            b01s = wpool.tile([128, M], F32, tag="b01")
            b1ps = wpool.tile([128, 2 * M], F32, tag="b1p")
            b1as = wpool.tile([128, M], F32, tag="b1a")
            b1os = wpool.tile([O1, M], F32, tag="b1o")

            x0s = acts.tile([128, ncol // 2], DT, tag="x0")
            x1s = acts.tile([128, ncol // 2], DT, tag="x1")
            h1a = acts.tile([128, ncol], DT, tag="h1a")
            h1b = acts.tile([128, ncol], DT, tag="h1b")
            hs = acts.tile([128, ncol], DT, tag="h")
            g1a = acts.tile([128, ncol], DT, tag="g1a")
            g1b = acts.tile([128, ncol], DT, tag="g1b")

            # DMA order = consumption order; x0 split per module so L1 i=0 can
            # start after ~0.7 MB instead of after the whole input set.
            nc.sync.dma_start(cast(w00s[:]), cast(w00))
            nc.sync.dma_start(b00s[:], b00)
            for i in range(M // 2):
                nc.sync.dma_start(cast(x0s[:, i * S:(i + 1) * S]),
                                  cast(x0t[:, i * S:(i + 1) * S]))
            nc.sync.dma_start(cast(w1ps[:]), cast(w1p))
            nc.sync.dma_start(b1ps[:], b1p)
            nc.sync.dma_start(cast(x1s[:]), cast(x1t))
            nc.sync.dma_start(cast(w01s[:]), cast(w01))
            nc.sync.dma_start(b01s[:], b01)
            nc.sync.dma_start(cast(w1as[:]), cast(w1a))
            nc.sync.dma_start(b1as[:], b1a)
            nc.sync.dma_start(cast(w1os[:]), cast(w1o))
            nc.sync.dma_start(b1os[:], b1o)

            parity = 0

            def evict(dst_ap, psum_ap, bias_ap, relu):
                nonlocal parity
                if relu and not USE_BF16:
                    # these tensors feed later fp32r matmuls -> write as fp32r
                    dst_ap = dst_ap.bitcast(F32R)
                if parity == 0:
                    if relu:
                        nc.scalar.activation(dst_ap, psum_ap, Relu, bias=bias_ap)
                    else:
                        nc.scalar.activation(dst_ap, psum_ap, Identity, bias=bias_ap)
                else:
                    if relu:
                        nc.vector.tensor_scalar(dst_ap, psum_ap, bias_ap, 0.0, Add, Max)
                    else:
                        nc.vector.tensor_scalar_add(dst_ap, psum_ap, bias_ap)
                parity ^= 1

            def strided(tensor_tile, j, b0, nb, cj):
                # columns off[j] + i*S + [0, cj) for i in [b0, b0+nb)
                v = tensor_tile[:].rearrange("p (i c) -> p i c", i=M)
                return v[:, b0:b0 + nb, off[j]:off[j] + cj]

            # ---- L1: h1 = relu(W00[a0].T @ x0 + b00[a0])  [H=256 -> 2 chunks]
            # modules ip and ip+4 run concurrently on PE row-halves (K=64 each)
            # two N-splits land in the two banks of one PSUM tile; one evict covers both
            def bank2(pt, sizes):
                v = pt[:].rearrange("p (b c) -> p b c", b=2)
                return [v[:, s, 0:sz] for s, (pos, sz) in enumerate(sizes)]

            for ip in range(M // 2):
                for mo in range(2):
                    dst_tile = h1a if mo == 0 else h1b
                    wcols = slice(ip * H + mo * 128, ip * H + (mo + 1) * 128)
                    for half, base, bia in ((slice(0, 64), ip, mo * M + ip),
                                            (slice(64, 128), ip + 4, mo * M + ip + 4)):
                        pt = ps128.tile([128, 1024], F32, tag="ps")
                        outs2 = bank2(pt, a0_splits)
                        for s, (pos, sz) in enumerate(a0_splits):
                            # both partition-halves read columns ip*S..: module
                            # ip+4's samples sit on partitions 64-127 at the
                            # same column offsets as module ip
                            nc.tensor.matmul(outs2[s], cast(w00s[half, wcols]),
                                             cast(x0s[half, ip * S + pos: ip * S + pos + sz]),
                                             start=True, stop=True)
                        sz0 = a0_splits[0][1]
                        src_ap = pt[:].rearrange("p (b c) -> p b c", b=2)[:, :, 0:sz0]
                        dst_ap = dst_tile[:, base * S: base * S + S].rearrange(
                            "p (b c) -> p b c", b=2)
                        evict(dst_ap, src_ap, b00s[:, bia: bia + 1], True)

            # ---- L3: g1 = relu(W1p[a1].T @ x1 + b1p[a1])  (emitted early: only needs x1)
            # block-halves i=0..3 / i=4..7 run concurrently on PE row-halves
            def strided_half(tile_t, part0, j, cj):
                v = tile_t[:].rearrange("p (i c) -> p i c", i=M // 2)
                return v[part0:part0 + 64, :, off[j]:off[j] + cj]

            for j in range(M):
                cj = Cj[j]
                for mo in range(2):
                    dst_tile = g1a if mo == 0 else g1b
                    wcols = slice(j * H + mo * 128, j * H + (mo + 1) * 128)
                    pt = ps128.tile([128, 1024], F32, tag="ps")
                    ptb = pt[:].rearrange("p (b c) -> p b c", b=2)
                    nc.tensor.matmul(ptb[:, 0, 0:4 * cj].rearrange("p (i c) -> p i c", c=cj),
                                     cast(w1ps[0:64, wcols]),
                                     cast(strided_half(x1s, 0, j, cj)),
                                     start=True, stop=True)
                    nc.tensor.matmul(ptb[:, 1, 0:4 * cj].rearrange("p (i c) -> p i c", c=cj),
                                     cast(w1ps[64:128, wcols]),
                                     cast(strided_half(x1s, 64, j, cj)),
                                     start=True, stop=True)
                    src_ap = ptb[:, :, 0:4 * cj].rearrange("p b (i c) -> p b i c", c=cj)
                    evict(strided(dst_tile, j, 0, 8, cj), src_ap,
                          b1ps[:, mo * M + j: mo * M + j + 1], True)

            # ---- L2: h = relu(W01[a0].T @ h1 + b01[a0])  [K=256 -> 2 accum chunks]
            for i in range(M):
                pt = ps128.tile([128, 1024], F32, tag="ps")
                outs2 = bank2(pt, a0_splits)
                for s, (pos, sz) in enumerate(a0_splits):
                    nc.tensor.matmul(
                        outs2[s],
                        cast(w01s[:, (0 * M + i) * O0: (0 * M + i + 1) * O0]),
                        cast(h1a[:, i * S + pos: i * S + pos + sz]),
                        start=True, stop=False)
                    nc.tensor.matmul(
                        outs2[s],
                        cast(w01s[:, (1 * M + i) * O0: (1 * M + i + 1) * O0]),
                        cast(h1b[:, i * S + pos: i * S + pos + sz]),
                        start=False, stop=True)
                sz0 = a0_splits[0][1]
                src_ap = pt[:].rearrange("p (b c) -> p b c", b=2)[:, :, 0:sz0]
                dst_ap = hs[:, i * S: i * S + S].rearrange("p (b c) -> p b c", b=2)
                evict(dst_ap, src_ap, b01s[:, i: i + 1], True)

            # ---- L4: g = relu(W1a[a1].T @ concat(h, g1) + b1a[a1]) [K=384 -> 3 accum chunks]
            # g reuses h1a's slot (h1 is dead after L2)
            gs = acts.tile([128, ncol], DT, tag="h1a")
            for j in range(M):
                cj = Cj[j]
                pt = ps128.tile([128, 1024], F32, tag="ps")
                ptb = pt[:].rearrange("p (b c) -> p b c", b=2)
                for s, (b0, nb) in enumerate(blk_splits):
                    ptv = ptb[:, s, 0:nb * cj].rearrange("p (i c) -> p i c", c=cj)
                    for kc, src in enumerate((hs, g1a, g1b)):
                        nc.tensor.matmul(
                            ptv,
                            cast(w1as[:, (kc * M + j) * O0: (kc * M + j + 1) * O0]),
                            cast(strided(src, j, b0, nb, cj)),
                            start=(kc == 0), stop=(kc == 2))
                src_ap = ptb[:, :, 0:4 * cj].rearrange("p b (i c) -> p b i c", c=cj)
                evict(strided(gs, j, 0, 8, cj), src_ap,
                      b1as[:, j: j + 1], True)

            # ---- L5: out = W1o[a1].T @ g + b1o[a1]  (identity)
            # outT reuses h1b's slot; written + DMA'd out per module j
            outs = acts.tile([O1, ncol], F32, tag="h1b")
            for j in range(M):
                cj = Cj[j]
                pt = ps128.tile([O1, 1024], F32, tag="ps")
                ptb = pt[:].rearrange("p (b c) -> p b c", b=2)
                for s, (b0, nb) in enumerate(blk_splits):
                    ptv = ptb[:, s, 0:nb * cj].rearrange("p (i c) -> p i c", c=cj)
                    nc.tensor.matmul(
                        ptv,
                        cast(w1os[:, j * O1: (j + 1) * O1]),
                        cast(strided(gs, j, b0, nb, cj)),
                        start=True, stop=True)
                src_ap = ptb[:, :, 0:4 * cj].rearrange("p b (i c) -> p b i c", c=cj)
                evict(strided(outs, j, 0, 8, cj), src_ap,
                      b1os[:, j: j + 1], False)
                # stream this module's output columns back while later modules compute
                vout = outs[:].rearrange("p (i c) -> p i c", i=M)[:, :, off[j]:off[j] + cj]
                vdst = outT.rearrange("p (i c) -> p i c", i=M)[:, :, off[j]:off[j] + cj]
                nc.sync.dma_start(vdst, vout)

    nc.compile()
    return nc


def _pack_weights(inputs):
    """Pack per-module weight stacks into SBUF-image layouts (contraction dim on
    partitions, [K<=128, chunks*modules*out] on the free axis)."""
    import ml_dtypes
    wdt = ml_dtypes.bfloat16 if USE_BF16 else np.float32
    f = lambda a: np.ascontiguousarray(a.astype(wdt))
    g = lambda a: np.ascontiguousarray(a.astype(np.float32))
    W00 = inputs["W00"]; W01 = inputs["W01"]; W1p = inputs["W1p"]
    W1a = inputs["W1a"]; W1o = inputs["W1o"]
    w00lo = W00[:M // 2].transpose(1, 0, 2).reshape(D0, M * H // 2)
    w00hi = W00[M // 2:].transpose(1, 0, 2).reshape(D0, M * H // 2)
    w1pp = W1p.transpose(1, 0, 2).reshape(D1, M * H)
    return {
        "w00": f(np.concatenate([w00lo, w00hi], axis=0)),
        "w01": f(W01.reshape(M, 2, 128, O0).transpose(2, 1, 0, 3).reshape(128, 2 * M * O0)),
        "w1p": f(np.concatenate([w1pp, w1pp], axis=0)),
        "w1a": f(W1a.reshape(M, 3, 128, O0).transpose(2, 1, 0, 3).reshape(128, 3 * M * O0)),
        "w1o": f(W1o.transpose(1, 0, 2).reshape(128, M * O1)),
        "b00": g(inputs["b00"].reshape(M, 2, 128).transpose(2, 1, 0).reshape(128, 2 * M)),
        "b01": g(inputs["b01"].T),
        "b1p": g(inputs["b1p"].reshape(M, 2, 128).transpose(2, 1, 0).reshape(128, 2 * M)),
        "b1a": g(inputs["b1a"].T),
        "b1o": g(inputs["b1o"].T),
    }


def _route(input_val):
    """Assign each sample to a (core, column) in the blocked layout."""
    a0 = np.argmax(input_val[:, D0 + D1: D0 + D1 + M], axis=1)
    a1 = np.argmax(input_val[:, D0 + D1 + M: D0 + D1 + 2 * M], axis=1)
    B = input_val.shape[0]
    nij = np.zeros((M, M), dtype=np.int64)
    np.add.at(nij, (a0, a1), 1)
    # capacities rounded up to even: fp32r matmul ifmap needs even innermost counts
    Cj = np.maximum((-(-nij.max(axis=0) // NCORES) + 1) // 2 * 2, 64)
    off = np.concatenate([[0], np.cumsum(Cj)[:-1]]).astype(np.int64)
    S = int(Cj.sum())
    ncol = M * S

    pairkey = a0 * M + a1
    order = np.argsort(pairkey, kind="stable")
    counts = np.bincount(pairkey, minlength=M * M)
    group_start = np.concatenate([[0], np.cumsum(counts)[:-1]])
    rank_sorted = np.arange(B) - np.repeat(group_start, counts)
    rank = np.empty(B, dtype=np.int64)
    rank[order] = rank_sorted
    core = rank % NCORES
    slot = rank // NCORES
    assert np.all(slot < Cj[a1]), "capacity overflow"
    col = a0 * S + off[a1] + slot
    return core, col, S, [int(c) for c in Cj], [int(o) for o in off], ncol


def kernel(**inputs):
    import os
    import ml_dtypes
    from concourse.bass_utils import run_bass_kernel_spmd

    input_val = np.asarray(inputs["input_val"], dtype=np.float32)
    B = input_val.shape[0]

    core, col, S, Cj, off, ncol = _route(input_val)

    xdt = ml_dtypes.bfloat16 if USE_BF16 else np.float32
    feat0 = input_val[:, :D0]
    feat1 = input_val[:, D0:D0 + D1]
    # split layout: module blocks i<4 (by a0) on partitions 0-63, i>=4 on 64-127
    half = ncol // 2
    X0T = np.zeros((NCORES, 128, half), dtype=xdt)
    X1T = np.zeros((NCORES, 128, half), dtype=xdt)
    hi = col >= half
    prow = np.where(hi, 64, 0)
    pcol = np.where(hi, col - half, col)
    for r in (0, 64):
        m = prow == r
        X0T[core[m], r:r + 64, pcol[m]] = feat0[m].astype(xdt)
        X1T[core[m], r:r + 64, pcol[m]] = feat1[m].astype(xdt)

    wmap = _pack_weights({k: np.asarray(v, dtype=np.float32) for k, v in inputs.items()
                          if k != "input_val"})

    nc = _build_bass(S, Cj, off, ncol)

    in_maps = [dict(wmap, x0t=np.ascontiguousarray(X0T[c]),
                    x1t=np.ascontiguousarray(X1T[c])) for c in range(NCORES)]
    res = run_bass_kernel_spmd(nc, in_maps, core_ids=list(range(NCORES)),
                               tmpdir=os.environ.get("BASS_TMPDIR"))
    global _LAST_RESULTS
    _LAST_RESULTS = res

    OUT = np.stack([r["outT"] for r in res.results])  # [NCORES, O1, ncol]
    return np.ascontiguousarray(OUT[core, :, col]).astype(np.float32)
